# revision 8
# baseline (speedup 1.0000x reference)
"""Trainium2 Bass kernel for a 4-layer dense transformer (nn_Athena_24739011625811).

Strategy (8 NeuronCores, one chip, SPMD):
  - Residual stream sequence-sharded: core c owns tokens [256c, 256c+256), kept
    S-major [s, e] in SBUF as f32. RMS norms are local per-partition reductions.
  - Attention head-sharded (tensor parallel): core c computes q-heads {2c, 2c+1}
    and kv-head c//2 for ALL tokens. Needs the full normalized activations ->
    AllGather of the local 256-token slice (E-major, bf16, 1MB/rank).
    Output projection uses the core's 256-row slice of wo -> partial [S, E]
    summed via ReduceScatter (chunked over E in 4 pieces) back to the
    sequence shard.
  - FFN hidden-sharded: core c owns 1024 of 8192 hidden units (gate+up cols,
    down rows). Same AllGather-in / ReduceScatter-out pattern.
  - LM head vocab-sharded: core c computes logits[:, 4000c:4000c+4000] for all
    tokens; host concatenates.
  - All matmuls in bf16 (f32 PSUM accumulation); residual + softmax sums f32.
  - Embedding gather happens on host (pure data movement) and is sharded as the
    initial residual.

The graph is identical on all 8 cores; only dram parameter contents differ.
"""

import math

import numpy as np
import ml_dtypes

import concourse.bass as bass
import concourse.mybir as mybir
import concourse.tile as tile
from concourse import bacc
from concourse.bass_utils import run_bass_kernel_spmd

BF16 = mybir.dt.bfloat16
F32 = mybir.dt.float32
AF = mybir.ActivationFunctionType
ALU = mybir.AluOpType

V, E, HID, L = 32000, 2048, 8192, 4
H, KV, DK = 16, 4, 128
S, WINDOW = 2048, 1024
EPS = 1e-5
NCORES = 8
SL = S // NCORES          # 256 tokens per core
NET = E // 128            # 16 e-tiles
NST = S // 128            # 16 global s-tiles
VSH = V // NCORES         # 4000 vocab per core
VCH = VSH // 8            # 500 per vocab chunk
HL = HID // NCORES        # 1024 hidden per core
RG = [list(range(NCORES))]

_BUILT = None


def build_graph(layers=L):
    nc = bacc.Bacc("TRN2", target_bir_lowering=False, debug=False, num_devices=NCORES)

    # ---- parameters (contents differ per core; shapes identical) ----
    x0_d = nc.declare_dram_parameter("x0", [SL, E], F32, isOutput=False)
    cos_d = nc.declare_dram_parameter("cosT", [128, S], F32, isOutput=False)
    sin_d = nc.declare_dram_parameter("sinT", [128, S], F32, isOutput=False)
    mask_d = nc.declare_dram_parameter("masks", [4, 128, 256], BF16, isOutput=False)
    ones_d = nc.declare_dram_parameter("ones", [128, 1], BF16, isOutput=False)
    ident_d = nc.declare_dram_parameter("ident", [128, 128], BF16, isOutput=False)
    wq_d = nc.declare_dram_parameter("wq", [layers, NET, 2, 128, 128], BF16, isOutput=False)
    wk_d = nc.declare_dram_parameter("wk", [layers, NET, 128, 128], BF16, isOutput=False)
    wv_d = nc.declare_dram_parameter("wv", [layers, NET, 128, 128], BF16, isOutput=False)
    wo_d = nc.declare_dram_parameter("wo", [layers, 2, 4, 128, 512], BF16, isOutput=False)
    wup_d = nc.declare_dram_parameter("wup", [layers, NET, 16, 128, 128], BF16, isOutput=False)
    wdn_d = nc.declare_dram_parameter("wdn", [layers, 8, 4, 128, 512], BF16, isOutput=False)
    wvoc_d = nc.declare_dram_parameter("wvoc", [NET, 8, 128, VCH], BF16, isOutput=False)
    out_d = nc.declare_dram_parameter("out", [S, VSH], F32, isOutput=True)

    inv_sqrt_dk = float(1.0 / math.sqrt(DK))

    with tile.TileContext(nc) as tc:
        from contextlib import ExitStack

        with ExitStack() as ctx:
            persist = ctx.enter_context(tc.tile_pool(name="persist", bufs=1))
            dcomm = ctx.enter_context(tc.tile_pool(name="dcomm", bufs=2, space="DRAM"))

            # residual (f32, S-major, 2 tiles) + constants
            x_sb = [persist.tile([128, E], F32, name=f"x{i}", tag=f"x{i}") for i in range(2)]
            for i in range(2):
                nc.sync.dma_start(out=x_sb[i][:], in_=x0_d[128 * i:128 * (i + 1), :])
            mask_sb = [persist.tile([128, 256], BF16, name=f"mask{i}", tag=f"mask{i}")
                       for i in range(4)]
            for i in range(4):
                nc.sync.dma_start(out=mask_sb[i][:], in_=mask_d[i, :, :])
            ones_sb = persist.tile([128, 1], BF16, name="ones", tag="ones")
            nc.sync.dma_start(out=ones_sb[:], in_=ones_d[:, :])
            ident_sb = persist.tile([128, 128], BF16, name="ident", tag="ident")
            nc.sync.dma_start(out=ident_sb[:], in_=ident_d[:, :])
            eps_sb = persist.tile([128, 1], F32, name="epsc", tag="epsc")
            nc.gpsimd.memset(eps_sb[:], float(EPS))

            def rmsnorm_ag(sbA, psA, tag):
                """norm local residual -> bf16 E-major -> DRAM -> AllGather.
                Returns ag_out dram tile [8, E, SL]."""
                ag_in = dcomm.tile([E, SL], BF16, name=f"agin_{tag}", tag="ag_in", bufs=2)
                for st in range(2):
                    sq = sbA.tile([128, E], BF16, name="sqd", tag="sqd", bufs=2)
                    ssum = sbA.tile([128, 1], F32, name="ssum", tag="ssum", bufs=2)
                    nc.scalar.activation(sq[:], x_sb[st][:], AF.Square, accum_out=ssum[:])
                    sq_s = sbA.tile([128, 1], F32, name="sqs", tag="sqs", bufs=2)
                    nc.scalar.activation(sq_s[:], ssum[:], AF.Sqrt,
                                         scale=float(1.0 / E), bias=eps_sb[:])
                    r = sbA.tile([128, 1], F32, name="rr", tag="rr", bufs=2)
                    nc.vector.reciprocal(r[:], sq_s[:])
                    n_s = sbA.tile([128, E], BF16, name="ns", tag="ns", bufs=2)
                    nc.scalar.mul(n_s[:], x_sb[st][:], r[:])
                    for et in range(NET):
                        pt = psA.tile([128, 128], BF16, name="tps", tag="tps", bufs=4)
                        nc.tensor.transpose(pt[:], n_s[:, et * 128:(et + 1) * 128], ident_sb[:])
                        stg = sbA.tile([128, 128], BF16, name="stg", tag="stg", bufs=4)
                        nc.scalar.copy(stg[:], pt[:])
                        nc.sync.dma_start(
                            out=ag_in[et * 128:(et + 1) * 128, st * 128:(st + 1) * 128],
                            in_=stg[:])
                ag_out = dcomm.tile([NCORES, E, SL], BF16, name=f"agout_{tag}",
                                    tag="ag_out", bufs=2, addr_space="Shared")
                nc.gpsimd.collective_compute(
                    "AllGather", ALU.bypass, replica_groups=RG,
                    ins=[ag_in[:].opt()], outs=[ag_out[:].opt()])
                return ag_out

            def residual_add(sbR, rs_outs):
                for st in range(2):
                    for ec in range(4):
                        d = sbR.tile([128, 512], BF16, name="delta", tag="delta", bufs=4)
                        nc.sync.dma_start(out=d[:],
                                          in_=rs_outs[ec][st * 128:(st + 1) * 128, :])
                        nc.vector.tensor_add(
                            x_sb[st][:, ec * 512:(ec + 1) * 512],
                            x_sb[st][:, ec * 512:(ec + 1) * 512], d[:])

            for l in range(layers):
                # ---------- attention ----------
                with tc.tile_pool(name=f"sbA_{l}", bufs=2) as sbA, \
                     tc.tile_pool(name=f"psA_{l}", bufs=2, space="PSUM") as psA:
                    ag1 = rmsnorm_ag(sbA, psA, f"n1_{l}")

                with tc.tile_pool(name=f"sbB_{l}", bufs=1) as sbB:
                    # qkv weights resident (small)
                    wq_sb = [[sbB.tile([128, 128], BF16, name="wqt", tag=f"wq{et}_{h}")
                              for h in range(2)] for et in range(NET)]
                    wk_sb = [sbB.tile([128, 128], BF16, name="wkt", tag=f"wk{et}")
                             for et in range(NET)]
                    wv_sb = [sbB.tile([128, 128], BF16, name="wvt", tag=f"wv{et}")
                             for et in range(NET)]
                    for et in range(NET):
                        for h in range(2):
                            nc.sync.dma_start(out=wq_sb[et][h][:], in_=wq_d[l, et, h, :, :])
                        nc.sync.dma_start(out=wk_sb[et][:], in_=wk_d[l, et, :, :])
                        nc.sync.dma_start(out=wv_sb[et][:], in_=wv_d[l, et, :, :])
                    cos_sb = sbB.tile([128, S], F32, name="cosb", tag="cosb")
                    sin_sb = sbB.tile([128, S], F32, name="sinb", tag="sinb")
                    nc.sync.dma_start(out=cos_sb[:], in_=cos_d[:, :])
                    nc.sync.dma_start(out=sin_sb[:], in_=sin_d[:, :])

                    q_sb = [sbB.tile([128, S], BF16, name="qh", tag=f"q{h}") for h in range(2)]
                    k_sb = sbB.tile([128, S], BF16, name="kh", tag="k")
                    v_sb = [sbB.tile([128, 128], BF16, name="vb", tag=f"v{jb}")
                            for jb in range(NST)]

                    def rope(ps, out_sl, sc, sbP):
                        cs = cos_sb[:, sc * 512:(sc + 1) * 512]
                        t0 = sbP.tile([128, 512], F32, name="rt0", tag="rt0", bufs=3)
                        nc.vector.tensor_mul(t0[:], ps[:], cs)
                        t1 = sbP.tile([128, 512], F32, name="rt1", tag="rt1", bufs=3)
                        nc.vector.tensor_mul(t1[0:64, :], ps[64:128, :],
                                             sin_sb[0:64, sc * 512:(sc + 1) * 512])
                        nc.vector.tensor_mul(t1[64:128, :], ps[0:64, :],
                                             sin_sb[64:128, sc * 512:(sc + 1) * 512])
                        nc.vector.tensor_add(out_sl, t0[:], t1[:])

                    psB_cm = tc.tile_pool(name=f"psB_{l}", bufs=2, space="PSUM")
                    psB = psB_cm.__enter__()
                    for sc in range(4):
                        nts = []
                        for et in range(NET):
                            nt = sbB.tile([128, 512], BF16, name="nt", tag=f"nt{et}", bufs=2)
                            nc.sync.dma_start(out=nt[:, 0:256],
                                              in_=ag1[2 * sc, et * 128:(et + 1) * 128, :])
                            nc.sync.dma_start(out=nt[:, 256:512],
                                              in_=ag1[2 * sc + 1, et * 128:(et + 1) * 128, :])
                            nts.append(nt)
                        for h in range(2):
                            psq = psB.tile([128, 512], F32, name="psq", tag="pqk", bufs=2)
                            for et in range(NET):
                                nc.tensor.matmul(psq[:], wq_sb[et][h][:], nts[et][:],
                                                 start=(et == 0), stop=(et == NET - 1))
                            rope(psq[:], q_sb[h][:, sc * 512:(sc + 1) * 512], sc, sbB)
                        psk = psB.tile([128, 512], F32, name="psk", tag="pqk", bufs=2)
                        for et in range(NET):
                            nc.tensor.matmul(psk[:], wk_sb[et][:], nts[et][:],
                                             start=(et == 0), stop=(et == NET - 1))
                        rope(psk[:], k_sb[:, sc * 512:(sc + 1) * 512], sc, sbB)
                        for b in range(4):
                            jb = 4 * sc + b
                            psv = psB.tile([128, 128], F32, name="psv", tag="psv", bufs=2)
                            for et in range(NET):
                                nc.tensor.matmul(psv[:],
                                                 nts[et][:, b * 128:(b + 1) * 128],
                                                 wv_sb[et][:],
                                                 start=(et == 0), stop=(et == NET - 1))
                            nc.scalar.copy(v_sb[jb][:], psv[:])
                    psB_cm.__exit__(None, None, None)

                    # ---- attention proper (transposed scores [keys, queries]) ----
                    with tc.tile_pool(name=f"sbC_{l}", bufs=1) as sbC:
                        psC_cm = tc.tile_pool(name=f"psC_{l}", bufs=2, space="PSUM")
                        psC = psC_cm.__enter__()
                        attnT = [sbC.tile([128, S], BF16, name="attnT", tag=f"attnT{h}")
                                 for h in range(2)]
                        for h in range(2):
                            for qp in range(8):
                                kb_lo = max(0, 2 * qp - 8)
                                kbs = list(range(kb_lo, 2 * qp + 2))
                                qsl = q_sb[h][:, qp * 256:(qp + 1) * 256]
                                pts = []
                                for kb in kbs:
                                    pss = psC.tile([128, 256], F32, name="pss",
                                                   tag="pss", bufs=3)
                                    nc.tensor.matmul(pss[:],
                                                     k_sb[:, kb * 128:(kb + 1) * 128],
                                                     qsl, start=True, stop=True)
                                    pt = sbC.tile([128, 256], BF16, name="pt",
                                                  tag="pt", bufs=12)
                                    nc.scalar.activation(pt[:], pss[:], AF.Exp,
                                                         scale=inv_sqrt_dk)
                                    mi = {1: 0, 0: 1, -7: 2, -8: 3}.get(kb - 2 * qp)
                                    if mi is not None:
                                        nc.vector.tensor_mul(pt[:], pt[:], mask_sb[mi][:])
                                    pts.append(pt)
                                psl = psC.tile([1, 256], F32, name="psl", tag="psl", bufs=1)
                                for i in range(len(kbs)):
                                    nc.tensor.matmul(psl[:], ones_sb[:], pts[i][:],
                                                     start=(i == 0), stop=(i == len(kbs) - 1))
                                psa = psC.tile([128, 256], F32, name="psa", tag="psa", bufs=2)
                                for i, kb in enumerate(kbs):
                                    nc.tensor.matmul(psa[:], v_sb[kb][:], pts[i][:],
                                                     start=(i == 0), stop=(i == len(kbs) - 1))
                                linv = sbC.tile([1, 256], F32, name="linv", tag="linv", bufs=2)
                                nc.vector.reciprocal(linv[:], psl[:])
                                lbc = sbC.tile([128, 256], F32, name="lbc", tag="lbc", bufs=2)
                                nc.gpsimd.partition_broadcast(lbc[:], linv[:])
                                nc.vector.tensor_mul(
                                    attnT[h][:, qp * 256:(qp + 1) * 256], psa[:], lbc[:])

                        psC_cm.__exit__(None, None, None)
                        # ---- output projection + chunked ReduceScatter ----
                        with tc.tile_pool(name=f"sbD_{l}", bufs=2) as sbD, \
                             tc.tile_pool(name=f"psD_{l}", bufs=2, space="PSUM") as psD:
                            rs_outs = []
                            for ec in range(4):
                                rs_in = dcomm.tile([S, 512], BF16, name="rsin",
                                                   tag="rs_in", bufs=4)
                                wo_sb = [sbD.tile([128, 512], BF16, name="wot",
                                                  tag=f"wo{ht}", bufs=2) for ht in range(2)]
                                for ht in range(2):
                                    nc.sync.dma_start(out=wo_sb[ht][:],
                                                      in_=wo_d[l, ht, ec, :, :])
                                for stg_i in range(NST):
                                    psy = psD.tile([128, 512], F32, name="psy",
                                                   tag="psy", bufs=3)
                                    for ht in range(2):
                                        nc.tensor.matmul(
                                            psy[:],
                                            attnT[ht][:, stg_i * 128:(stg_i + 1) * 128],
                                            wo_sb[ht][:],
                                            start=(ht == 0), stop=(ht == 1))
                                    ysb = sbD.tile([128, 512], BF16, name="ysb",
                                                   tag="ysb", bufs=4)
                                    nc.scalar.copy(ysb[:], psy[:])
                                    nc.sync.dma_start(
                                        out=rs_in[stg_i * 128:(stg_i + 1) * 128, :],
                                        in_=ysb[:])
                                rs_out = dcomm.tile([SL, 512], BF16, name="rsout",
                                                    tag="rs_out", bufs=4)
                                nc.gpsimd.collective_compute(
                                    "ReduceScatter", ALU.add, replica_groups=RG,
                                    ins=[rs_in[:].opt()], outs=[rs_out[:].opt()])
                                rs_outs.append(rs_out)
                            residual_add(sbD, rs_outs)

                # ---------- FFN ----------
                with tc.tile_pool(name=f"sbA2_{l}", bufs=2) as sbA2, \
                     tc.tile_pool(name=f"psA2_{l}", bufs=2, space="PSUM") as psA2:
                    ag2 = rmsnorm_ag(sbA2, psA2, f"n2_{l}")

                with tc.tile_pool(name=f"sbF_{l}", bufs=1) as sbF, \
                     tc.tile_pool(name=f"psF_{l}", bufs=2, space="PSUM") as psF:
                    n2 = [sbF.tile([128, S], BF16, name="n2t", tag=f"n2_{et}")
                          for et in range(NET)]
                    for et in range(NET):
                        for r in range(NCORES):
                            nc.sync.dma_start(
                                out=n2[et][:, r * SL:(r + 1) * SL],
                                in_=ag2[r, et * 128:(et + 1) * 128, :])
                    hid = [sbF.tile([128, S], BF16, name="hidt", tag=f"hid{hc}")
                           for hc in range(8)]
                    for hcp in range(8):
                        wg_sb = [sbF.tile([128, 128], BF16, name="wgt",
                                          tag=f"wg{et}", bufs=2) for et in range(NET)]
                        wu_sb = [sbF.tile([128, 128], BF16, name="wut",
                                          tag=f"wu{et}", bufs=2) for et in range(NET)]
                        for et in range(NET):
                            nc.sync.dma_start(out=wg_sb[et][:], in_=wup_d[l, et, hcp, :, :])
                            nc.sync.dma_start(out=wu_sb[et][:],
                                              in_=wup_d[l, et, 8 + hcp, :, :])
                        for sc in range(4):
                            ssl = slice(sc * 512, (sc + 1) * 512)
                            psg = psF.tile([128, 512], F32, name="psg", tag="psg", bufs=2)
                            for et in range(NET):
                                nc.tensor.matmul(psg[:], wg_sb[et][:], n2[et][:, ssl],
                                                 start=(et == 0), stop=(et == NET - 1))
                            sg = sbF.tile([128, 512], BF16, name="sg", tag="sg", bufs=3)
                            nc.scalar.activation(sg[:], psg[:], AF.Silu)
                            psu = psF.tile([128, 512], F32, name="psu", tag="psu", bufs=2)
                            for et in range(NET):
                                nc.tensor.matmul(psu[:], wu_sb[et][:], n2[et][:, ssl],
                                                 start=(et == 0), stop=(et == NET - 1))
                            nc.vector.tensor_mul(hid[hcp][:, ssl], psu[:], sg[:])
                    rs_outs = []
                    for ec in range(4):
                        rs_in = dcomm.tile([S, 512], BF16, name="rsin2",
                                           tag="rs_in", bufs=4)
                        wd_sb = [sbF.tile([128, 512], BF16, name="wdt",
                                          tag=f"wd{ht}", bufs=2) for ht in range(8)]
                        for ht in range(8):
                            nc.sync.dma_start(out=wd_sb[ht][:], in_=wdn_d[l, ht, ec, :, :])
                        for stg_i in range(NST):
                            psy = psF.tile([128, 512], F32, name="psy2", tag="psy2", bufs=3)
                            for ht in range(8):
                                nc.tensor.matmul(
                                    psy[:], hid[ht][:, stg_i * 128:(stg_i + 1) * 128],
                                    wd_sb[ht][:], start=(ht == 0), stop=(ht == 7))
                            ysb = sbF.tile([128, 512], BF16, name="ysb2", tag="ysb2", bufs=4)
                            nc.scalar.copy(ysb[:], psy[:])
                            nc.sync.dma_start(
                                out=rs_in[stg_i * 128:(stg_i + 1) * 128, :], in_=ysb[:])
                        rs_out = dcomm.tile([SL, 512], BF16, name="rsout2",
                                            tag="rs_out", bufs=4)
                        nc.gpsimd.collective_compute(
                            "ReduceScatter", ALU.add, replica_groups=RG,
                            ins=[rs_in[:].opt()], outs=[rs_out[:].opt()])
                        rs_outs.append(rs_out)
                    residual_add(sbF, rs_outs)

            # ---------- final norm + vocab projection ----------
            with tc.tile_pool(name="sbAF", bufs=2) as sbAF, \
                 tc.tile_pool(name="psAF", bufs=2, space="PSUM") as psAF:
                ag3 = rmsnorm_ag(sbAF, psAF, "nf")

            with tc.tile_pool(name="sbV", bufs=1) as sbV, \
                 tc.tile_pool(name="psV", bufs=2, space="PSUM") as psV:
                nf = [sbV.tile([128, S], BF16, name="nft", tag=f"nf{et}")
                      for et in range(NET)]
                for et in range(NET):
                    for r in range(NCORES):
                        nc.sync.dma_start(out=nf[et][:, r * SL:(r + 1) * SL],
                                          in_=ag3[r, et * 128:(et + 1) * 128, :])
                for vc in range(8):
                    wvt = [sbV.tile([128, VCH], BF16, name="wvct",
                                    tag=f"wvc{et}", bufs=2) for et in range(NET)]
                    for et in range(NET):
                        nc.sync.dma_start(out=wvt[et][:], in_=wvoc_d[et, vc, :, :])
                    for stg_i in range(NST):
                        psv = psV.tile([128, VCH], F32, name="psvv", tag="psvv", bufs=4)
                        for et in range(NET):
                            nc.tensor.matmul(
                                psv[:], nf[et][:, stg_i * 128:(stg_i + 1) * 128],
                                wvt[et][:], start=(et == 0), stop=(et == NET - 1))
                        osb = sbV.tile([128, VCH], F32, name="osb", tag="osb", bufs=4)
                        nc.scalar.copy(osb[:], psv[:])
                        nc.sync.dma_start(
                            out=out_d[stg_i * 128:(stg_i + 1) * 128,
                                      vc * VCH:(vc + 1) * VCH],
                            in_=osb[:])

    nc.compile()
    return nc


# ------------------------------------------------------------------ host side

def _bf16(a):
    return np.ascontiguousarray(a).astype(ml_dtypes.bfloat16)


def _prepare_inmaps(tokens, table, wq, wk, wv, wo, w_up, w_down, w_vocab, layers=L):
    tokens = np.asarray(tokens)
    table = np.asarray(table, dtype=np.float32)
    wq = np.asarray(wq, dtype=np.float32)
    wk = np.asarray(wk, dtype=np.float32)
    wv = np.asarray(wv, dtype=np.float32)
    wo = np.asarray(wo, dtype=np.float32)
    w_up = np.asarray(w_up, dtype=np.float32)
    w_down = np.asarray(w_down, dtype=np.float32)
    w_vocab = np.asarray(w_vocab, dtype=np.float32)

    tbl = table.copy()
    tbl[0] = 0.0
    x_full = tbl[tokens[0]]  # [S, E] f32

    # rope tables, d-major [DK, S], sign-flip folded into sin
    half = DK // 2
    offs = np.arange(DK) % half
    scales = np.power(10000.0, -2.0 / DK * offs.astype(np.float64))
    ang = np.arange(S, dtype=np.float64)[:, None] * scales[None, :]  # [S, DK]
    cosT = np.cos(ang).T.astype(np.float32).copy()                   # [DK, S]
    sinT = np.sin(ang).T.astype(np.float32)
    sinT[:half, :] *= -1.0
    sinT = sinT.copy()

    # masks (transposed coords [j, i]); halves are the two q-tiles of a 256 pair
    jj = np.arange(128)[:, None]
    ii = np.arange(128)[None, :]
    causal = (jj <= ii).astype(np.float32)   # same-block visibility
    anti = (jj > ii).astype(np.float32)      # far-window block
    full = np.ones((128, 128), np.float32)
    zero = np.zeros((128, 128), np.float32)
    masks = np.stack([
        np.concatenate([zero, causal], axis=1),   # rel +1
        np.concatenate([causal, full], axis=1),   # rel 0
        np.concatenate([full, anti], axis=1),     # rel -7
        np.concatenate([anti, zero], axis=1),     # rel -8
    ])  # [4, 128, 256]

    ident = np.eye(128, dtype=np.float32)
    ones = np.ones((128, 1), np.float32)

    in_maps = []
    for c in range(NCORES):
        g = c // 2
        wq_c = wq[:layers, :, 256 * c:256 * c + 256]
        wq_c = wq_c.reshape(layers, NET, 128, 2, 128).transpose(0, 1, 3, 2, 4)
        wk_c = wk[:layers, :, 128 * g:128 * g + 128].reshape(layers, NET, 128, 128)
        wv_c = wv[:layers, :, 128 * g:128 * g + 128].reshape(layers, NET, 128, 128)
        wo_c = wo[:layers, 256 * c:256 * c + 256, :]
        wo_c = wo_c.reshape(layers, 2, 128, 4, 512).transpose(0, 1, 3, 2, 4)
        gate_c = w_up[:layers, :, HL * c:HL * c + HL]
        up_c = w_up[:layers, :, HID + HL * c:HID + HL * c + HL]
        gate_c = gate_c.reshape(layers, NET, 128, 8, 128).transpose(0, 1, 3, 2, 4)
        up_c = up_c.reshape(layers, NET, 128, 8, 128).transpose(0, 1, 3, 2, 4)
        wup_c = np.concatenate([gate_c, up_c], axis=2)  # [L, NET, 16, 128, 128]
        wdn_c = w_down[:layers, HL * c:HL * c + HL, :]
        wdn_c = wdn_c.reshape(layers, 8, 128, 4, 512).transpose(0, 1, 3, 2, 4)
        wvoc_c = w_vocab[:, VSH * c:VSH * c + VSH]
        wvoc_c = wvoc_c.reshape(NET, 128, 8, VCH).transpose(0, 2, 1, 3)

        in_maps.append({
            "x0": np.ascontiguousarray(x_full[SL * c:SL * c + SL]),
            "cosT": cosT,
            "sinT": sinT,
            "masks": _bf16(masks),
            "ones": _bf16(ones),
            "ident": _bf16(ident),
            "wq": _bf16(wq_c),
            "wk": _bf16(wk_c),
            "wv": _bf16(wv_c),
            "wo": _bf16(wo_c),
            "wup": _bf16(wup_c),
            "wdn": _bf16(wdn_c),
            "wvoc": _bf16(wvoc_c),
        })
    return in_maps


def _run(inputs, trace=False, layers=L):
    global _BUILT
    if _BUILT is None or _BUILT[1] != layers:
        _BUILT = (build_graph(layers), layers)
    nc = _BUILT[0]
    in_maps = _prepare_inmaps(layers=layers, **inputs)
    res = run_bass_kernel_spmd(nc, in_maps, core_ids=list(range(NCORES)), trace=trace)
    logits = np.concatenate([res.results[c]["out"] for c in range(NCORES)], axis=1)
    return logits[None].astype(np.float32), res


def kernel(**inputs):
    logits, _ = _run(inputs, trace=False)
    return logits
